# revision 4
# baseline (speedup 1.0000x reference)
"""Trainium2 Bass kernel for a 2-layer GAT (nn_GAT_1236950581751).

Strategy (8 NeuronCores, SPMD, one program):
  - Nodes are sharded contiguously: core c owns nodes [c*12500, (c+1)*12500),
    locally reordered by in-degree (descending) so that 128-node dst tiles
    have near-uniform degree.
  - Host folds weights: layer-1 needs only h1pre = x @ (W_lin@W1) + b_lin@W1
    plus the 4 attention projections -> one [768, 68] matmul per node.
  - Device: fm = Wbig^T @ x^T (feature-major), PE-transpose to node-major,
    write a local node table [12500, 66] (h1pre + alpha_src), AllGather to the
    full table [100001, 66] (row 100000 is a dummy row used for ELL padding:
    h = 0, alpha_src = -300 so exp() underflows to 0).
  - Per dst tile (128 nodes, ELL with D_t slot-columns): one indirect DMA per
    slot column gathers table[idx[p, j]] into SBUF; DVE computes the
    segment-softmax (max-shifted exp) and the alpha-weighted sum along the
    slot axis; bias add; PE-transpose into a feature-major h1 buffer.
  - Layer 2 repeats the same structure with a [100001, 4] table
    (h2pre(3) + alpha_src2) and 1 head; output written per tile.
  - Host un-permutes the 8 output shards into the full [100000, 3] result.
"""

import numpy as np

N = 100000
C = 8                 # cores
S = N // C            # 12500 nodes per shard
P = 128
NT = (S + P - 1) // P  # 98 tiles per core
S_PAD = NT * P         # 12544
DUMMY = N              # dummy table row
TR = N + 1             # table rows
RC1 = 66               # layer-1 table row: h1pre(64) + alpha_src(2)
RC2 = 4                # layer-2 table row: h2pre(3) + alpha_src2(1)
ALPHA_PAD = -300.0
NEG_SLOPE = 0.2
H = 2                  # layer-1 heads
CH = 32                # channels per head
F1 = 68                # fm channels: 64 h1pre + 2 a_src + 2 a_dst
F2 = 5                 # fm2 channels: 3 h2pre + 1 a_src + 1 a_dst
KX = 6                 # 768 / 128 contraction chunks
NCHUNK = 512           # matmul free-dim tile

_CACHE = {}


def _fold_weights(W_lin, b_lin, W1, att_src1, att_dst1, W2, att_src2, att_dst2):
    Wf = (W_lin.astype(np.float64) @ W1.astype(np.float64))
    bf = (b_lin.astype(np.float64) @ W1.astype(np.float64))
    cols = [Wf]
    bb = [bf]
    for att in (att_src1, att_dst1):
        for h in range(H):
            a = att[h].astype(np.float64)
            cols.append((Wf[:, CH * h:CH * (h + 1)] @ a)[:, None])
            bb.append(np.array([bf[CH * h:CH * (h + 1)] @ a]))
    Wbig = np.concatenate(cols, axis=1).astype(np.float32)        # [768, 68]
    bbig = np.concatenate(bb).astype(np.float32)                  # [68]
    W2l = W2.astype(np.float64)
    P2 = np.concatenate(
        [W2l, (W2l @ att_src2[0].astype(np.float64))[:, None],
         (W2l @ att_dst2[0].astype(np.float64))[:, None]], axis=1
    ).astype(np.float32)                                          # [64, 5]
    return Wbig, bbig, P2


def _preprocess(edge_index):
    """Static graph preprocessing -> per-core idx arrays + shared schedule."""
    src = np.concatenate([np.asarray(edge_index[0], dtype=np.int64),
                          np.arange(N, dtype=np.int64)])
    dst = np.concatenate([np.asarray(edge_index[1], dtype=np.int64),
                          np.arange(N, dtype=np.int64)])
    deg = np.bincount(dst, minlength=N).astype(np.int64)

    # CSR over dst
    order_e = np.argsort(dst, kind="stable")
    src_by_dst = src[order_e]
    rowptr = np.zeros(N + 1, np.int64)
    rowptr[1:] = np.cumsum(deg)

    # per-shard degree-descending node order
    orders = np.empty((C, S), np.int64)
    for c in range(C):
        nodes = np.arange(c * S, (c + 1) * S)
        orders[c] = nodes[np.argsort(-deg[nodes], kind="stable")]
    rank = np.empty(N, np.int64)
    for c in range(C):
        rank[orders[c]] = np.arange(S)
    shard_of = np.arange(N) // S

    # chunk-major table slot numbering (4 collective chunks, tile aligned)
    chunk_tiles = [25, 25, 25, 23]
    starts = np.array([0, 3200, 6400, 9600], np.int64)
    sizes = np.array([3200, 3200, 3200, 2900], np.int64)
    bases = np.array([0, 25600, 51200, 76800], np.int64)
    j_of = np.minimum(rank // 3200, 3)
    slot = bases[j_of] + shard_of * sizes[j_of] + (rank - starts[j_of])

    # shared per-tile max-degree schedule
    Dt = np.zeros(NT, np.int64)
    for c in range(C):
        dpad = np.zeros(S_PAD, np.int64)
        dpad[:S] = deg[orders[c]]
        Dt = np.maximum(Dt, dpad.reshape(NT, P).max(1))
    Dt = np.maximum(Dt, 1).astype(np.int64)
    toff = np.zeros(NT + 1, np.int64)
    toff[1:] = np.cumsum(Dt)
    TOT = int(toff[-1])

    # per-core ELL index array [128, TOT] int32 (slot ids; DUMMY padding)
    Dmax = int(Dt.max())
    col = np.arange(Dmax)[None, :]
    idx_cores = []
    for c in range(C):
        nodes = orders[c]
        counts = deg[nodes]
        ell = np.full((S_PAD, Dmax), DUMMY, np.int32)
        mask = col < counts[:, None]
        pos = (rowptr[nodes][:, None] + col)[mask]
        rr, cc = np.nonzero(mask)
        ell[rr, cc] = slot[src_by_dst[pos]].astype(np.int32)
        idxc = np.empty((P, TOT), np.int32)
        for t in range(NT):
            idxc[:, toff[t]:toff[t + 1]] = ell[t * P:(t + 1) * P, :Dt[t]]
        idx_cores.append(idxc)

    sched = {
        "Dt": [int(d) for d in Dt],
        "toff": [int(o) for o in toff],
        "TOT": TOT,
        "chunk_tiles": chunk_tiles,
        "chunk_rows": [int(x) for x in sizes],
        "chunk_starts": [int(x) for x in starts],
        "chunk_bases": [int(x) for x in bases],
    }
    return orders, idx_cores, sched


def _build_program(sched):
    import concourse.bass as bass
    import concourse.mybir as mybir
    import concourse.tile as tile
    from concourse import bacc
    from concourse.masks import make_identity

    f32 = mybir.dt.float32
    i32 = mybir.dt.int32
    Dt = sched["Dt"]
    toff = sched["toff"]
    TOT = sched["TOT"]

    nc = bacc.Bacc("TRN2", target_bir_lowering=False, debug=False,
                   enable_asserts=False, num_devices=C)

    xT = nc.dram_tensor("xT", [768, S_PAD], f32, kind="ExternalInput")
    Wbig_d = nc.dram_tensor("Wbig", [768, F1], f32, kind="ExternalInput")
    bbig_d = nc.dram_tensor("bbig", [F1, 1], f32, kind="ExternalInput")
    P2_d = nc.dram_tensor("P2", [64, F2], f32, kind="ExternalInput")
    b1_d = nc.dram_tensor("b1", [64], f32, kind="ExternalInput")
    b2_d = nc.dram_tensor("b2", [3], f32, kind="ExternalInput")
    idx_d = nc.dram_tensor("idx", [P, TOT], i32, kind="ExternalInput")
    out_d = nc.dram_tensor("out", [S, 3], f32, kind="ExternalOutput")

    tb1_loc = nc.dram_tensor("tb1_loc", [S, RC1], f32, kind="Internal")
    tb1_full = nc.dram_tensor("tb1_full", [TR, RC1], f32, kind="Internal",
                              addr_space="Shared")
    tb2_loc = nc.dram_tensor("tb2_loc", [S, RC2], f32, kind="Internal")
    tb2_full = nc.dram_tensor("tb2_full", [TR, RC2], f32, kind="Internal",
                              addr_space="Shared")

    # matmul N chunks
    chunks = []
    c0 = 0
    while c0 < S_PAD:
        cw = min(NCHUNK, S_PAD - c0)
        chunks.append((c0, cw))
        c0 += cw

    from contextlib import ExitStack

    with tile.TileContext(nc) as tc, ExitStack() as stack:
        const = stack.enter_context(tc.tile_pool(name="const", bufs=1))
        big = stack.enter_context(tc.tile_pool(name="big", bufs=1))
        io = stack.enter_context(tc.tile_pool(name="io", bufs=3))
        fmp = stack.enter_context(tc.tile_pool(name="fmp", bufs=2))
        nmp = stack.enter_context(tc.tile_pool(name="nmp", bufs=3))
        gp = stack.enter_context(tc.tile_pool(name="gp", bufs=4))
        wk = stack.enter_context(tc.tile_pool(name="wk", bufs=4))
        ps = stack.enter_context(tc.tile_pool(name="ps", bufs=2, space="PSUM"))

        # ---- constants ----
        wtiles = const.tile([P, KX, F1], f32)
        for k in range(KX):
            nc.sync.dma_start(out=wtiles[:, k, :], in_=Wbig_d[k * P:(k + 1) * P, :])
        bbig_sb = const.tile([F1, 1], f32)
        nc.sync.dma_start(out=bbig_sb[:], in_=bbig_d[:])
        p2_sb = const.tile([64, F2], f32)
        nc.sync.dma_start(out=p2_sb[:], in_=P2_d[:])
        b1_bc = const.tile([P, 64], f32)
        nc.sync.dma_start(out=b1_bc[:], in_=bass.AP(
            tensor=b1_d, offset=0, ap=[[0, P], [1, 64]]))
        b2_bc = const.tile([P, 3], f32)
        nc.sync.dma_start(out=b2_bc[:], in_=bass.AP(
            tensor=b2_d, offset=0, ap=[[0, P], [1, 3]]))
        id68 = const.tile([F1, F1], f32)
        make_identity(nc, id68[:])
        id128 = const.tile([P, P], f32)
        make_identity(nc, id128[:])
        id5 = const.tile([F2, F2], f32)
        make_identity(nc, id5[:])

        idx_all = big.tile([P, TOT], i32)
        nc.sync.dma_start(out=idx_all[:], in_=idx_d[:])
        h1T_all = big.tile([64, S_PAD], f32)
        aD1 = big.tile([P, 2 * NT], f32)
        aD2 = big.tile([P, NT], f32)

        # dummy rows
        dummy1 = const.tile([1, RC1], f32)
        nc.vector.memset(dummy1[:], 0.0)
        nc.vector.memset(dummy1[:, 64:66], ALPHA_PAD)
        nc.sync.dma_start(out=tb1_full[DUMMY:DUMMY + 1, :], in_=dummy1[:])
        dummy2 = const.tile([1, RC2], f32)
        nc.vector.memset(dummy2[:], 0.0)
        nc.vector.memset(dummy2[:, 3:4], ALPHA_PAD)
        nc.sync.dma_start(out=tb2_full[DUMMY:DUMMY + 1, :], in_=dummy2[:])

        # ---- STEP A: fm = Wbig^T @ x^T, transpose, write local table ----
        t_idx = 0
        for (cst, cw) in chunks:
            ps_fm = ps.tile([F1, cw], f32, tag="fm")
            for k in range(KX):
                xt = io.tile([P, cw], f32, tag="xt")
                nc.sync.dma_start(out=xt[:], in_=xT[k * P:(k + 1) * P, cst:cst + cw])
                nc.tensor.matmul(out=ps_fm[:], lhsT=wtiles[:, k, :], rhs=xt[:],
                                 start=(k == 0), stop=(k == KX - 1))
            fm_sb = fmp.tile([F1, cw], f32, tag="fm_sb")
            nc.vector.tensor_scalar(out=fm_sb[:], in0=ps_fm[:],
                                    scalar1=bbig_sb[:, 0:1], scalar2=None,
                                    op0=mybir.AluOpType.add)
            for sub in range(cw // P):
                t = t_idx
                t_idx += 1
                ps_tr = ps.tile([P, F1], f32, tag="tr")
                nc.tensor.transpose(out=ps_tr[:], in_=fm_sb[:, sub * P:(sub + 1) * P],
                                    identity=id68[:])
                nm = nmp.tile([P, F1], f32, tag="nm")
                nc.vector.tensor_copy(out=nm[:], in_=ps_tr[:])
                rows = min(P, S - t * P)
                if rows > 0:
                    nc.sync.dma_start(out=tb1_loc[t * P:t * P + rows, :],
                                      in_=nm[:rows, 0:RC1])
                nc.vector.tensor_copy(out=aD1[:, 2 * t:2 * t + 2], in_=nm[:, 66:68])

        # ---- STEP B: chunked AllGather of table1 ----
        for j in range(4):
            st = sched["chunk_starts"][j]
            sz = sched["chunk_rows"][j]
            bs = sched["chunk_bases"][j]
            nc.gpsimd.collective_compute(
                "AllGather", mybir.AluOpType.bypass,
                replica_groups=[list(range(C))],
                ins=[tb1_loc[st:st + sz, :].opt()],
                outs=[tb1_full[bs:bs + C * sz, :].opt()],
            )

        # ---- STEP C: layer-1 aggregation per tile ----
        def gat_tile(t, tbl, rc, nch, aD, out_cb):
            """Gather + segment softmax + weighted sum for dst tile t."""
            D = Dt[t]
            g = gp.tile([P, D * rc], f32, tag=f"g{rc}",
                        padded_shape=[P, max(Dt) * rc])
            for j in range(D):
                nc.gpsimd.indirect_dma_start(
                    out=g[:, j * rc:(j + 1) * rc],
                    out_offset=None,
                    in_=tbl[:],
                    in_offset=bass.IndirectOffsetOnAxis(
                        ap=idx_all[:, toff[t] + j:toff[t] + j + 1], axis=0),
                )
            g3 = g[:].rearrange("p (d c) -> p d c", c=rc)
            if nch == 64:  # layer 1, H=2 heads
                asv = g3[:, :, 64:66].transpose([0, 2, 1])        # [P,2,D]
                adv = aD[:, 2 * t:2 * t + 2].unsqueeze(2).to_broadcast([P, 2, D])
                e = wk.tile([P, 2, D], f32, tag="e")
                nc.vector.tensor_tensor(out=e[:], in0=asv, in1=adv,
                                        op=mybir.AluOpType.add)
                tmp = wk.tile([P, 2, D], f32, tag="tmp")
                nc.vector.tensor_scalar_mul(tmp[:], e[:], NEG_SLOPE)
                nc.vector.tensor_tensor(out=e[:], in0=e[:], in1=tmp[:],
                                        op=mybir.AluOpType.max)
                negm = wk.tile([P, 2], f32, tag="negm")
                nc.vector.tensor_reduce(out=negm[:], in_=e[:],
                                        axis=mybir.AxisListType.X,
                                        op=mybir.AluOpType.max, negate=True)
                pp = wk.tile([P, 2, D], f32, tag="pp")
                for h in range(2):
                    nc.scalar.activation(out=pp[:, h, :], in_=e[:, h, :],
                                         func=mybir.ActivationFunctionType.Exp,
                                         bias=negm[:, h:h + 1], scale=1.0)
                den = wk.tile([P, 2], f32, tag="den")
                nc.vector.tensor_reduce(out=den[:], in_=pp[:],
                                        axis=mybir.AxisListType.X,
                                        op=mybir.AluOpType.add)
                inv = wk.tile([P, 2], f32, tag="inv")
                nc.vector.reciprocal(inv[:], den[:])
                hv = g3[:, :, 0:64].rearrange("p d (h c) -> p d h c", h=2)
                pv = pp[:].transpose([0, 2, 1]).unsqueeze(3).to_broadcast(
                    [P, D, 2, CH])
                nc.vector.tensor_tensor(out=hv, in0=hv, in1=pv,
                                        op=mybir.AluOpType.mult)
                o = wk.tile([P, 2, CH], f32, tag="o")
                nc.vector.tensor_reduce(out=o[:], in_=hv.transpose([0, 2, 3, 1]),
                                        axis=mybir.AxisListType.X,
                                        op=mybir.AluOpType.add)
                invv = inv[:].unsqueeze(2).to_broadcast([P, 2, CH])
                nc.vector.tensor_tensor(out=o[:], in0=o[:], in1=invv,
                                        op=mybir.AluOpType.mult)
                h1 = wk.tile([P, 64], f32, tag="h1")
                nc.vector.tensor_tensor(out=h1[:], in0=o[:].rearrange("p h c -> p (h c)"),
                                        in1=b1_bc[:], op=mybir.AluOpType.add)
                out_cb(t, h1)
            else:  # layer 2, 1 head, 3 channels
                as2 = g3[:, :, 3:4].squeeze(2)                    # [P, D]
                e = wk.tile([P, D], f32, tag="e2")
                nc.vector.tensor_scalar(out=e[:], in0=as2, scalar1=aD[:, t:t + 1],
                                        scalar2=None, op0=mybir.AluOpType.add)
                tmp = wk.tile([P, D], f32, tag="tmp2")
                nc.vector.tensor_scalar_mul(tmp[:], e[:], NEG_SLOPE)
                nc.vector.tensor_tensor(out=e[:], in0=e[:], in1=tmp[:],
                                        op=mybir.AluOpType.max)
                negm = wk.tile([P, 1], f32, tag="negm2")
                nc.vector.tensor_reduce(out=negm[:], in_=e[:],
                                        axis=mybir.AxisListType.X,
                                        op=mybir.AluOpType.max, negate=True)
                pp = wk.tile([P, D], f32, tag="pp2")
                nc.scalar.activation(out=pp[:], in_=e[:],
                                     func=mybir.ActivationFunctionType.Exp,
                                     bias=negm[:, 0:1], scale=1.0)
                den = wk.tile([P, 1], f32, tag="den2")
                nc.vector.tensor_reduce(out=den[:], in_=pp[:],
                                        axis=mybir.AxisListType.X,
                                        op=mybir.AluOpType.add)
                inv = wk.tile([P, 1], f32, tag="inv2")
                nc.vector.reciprocal(inv[:], den[:])
                hv = g3[:, :, 0:3]
                pv = pp[:].unsqueeze(2).to_broadcast([P, D, 3])
                nc.vector.tensor_tensor(out=hv, in0=hv, in1=pv,
                                        op=mybir.AluOpType.mult)
                o = wk.tile([P, 3], f32, tag="o2")
                nc.vector.tensor_reduce(out=o[:], in_=hv.transpose([0, 2, 1]),
                                        axis=mybir.AxisListType.X,
                                        op=mybir.AluOpType.add)
                nc.vector.tensor_scalar(out=o[:], in0=o[:], scalar1=inv[:, 0:1],
                                        scalar2=None, op0=mybir.AluOpType.mult)
                nc.vector.tensor_tensor(out=o[:], in0=o[:], in1=b2_bc[:],
                                        op=mybir.AluOpType.add)
                out_cb(t, o)

        def l1_out(t, h1):
            ps_h1t = ps.tile([64, P], f32, tag="h1t")
            nc.tensor.transpose(out=ps_h1t[:], in_=h1[:], identity=id128[:])
            nc.vector.tensor_copy(out=h1T_all[:, t * P:(t + 1) * P], in_=ps_h1t[:])

        for t in range(NT):
            gat_tile(t, tb1_full, RC1, 64, aD1, l1_out)

        # ---- STEP D: layer-2 node projections + table2 ----
        t_idx = 0
        for (cst, cw) in chunks:
            ps2 = ps.tile([F2, cw], f32, tag="fm")
            nc.tensor.matmul(out=ps2[:], lhsT=p2_sb[:], rhs=h1T_all[:, cst:cst + cw],
                             start=True, stop=True)
            fm2 = fmp.tile([F2, cw], f32, tag="fm2_sb")
            nc.vector.tensor_copy(out=fm2[:], in_=ps2[:])
            for sub in range(cw // P):
                t = t_idx
                t_idx += 1
                ps_tr2 = ps.tile([P, F2], f32, tag="tr")
                nc.tensor.transpose(out=ps_tr2[:], in_=fm2[:, sub * P:(sub + 1) * P],
                                    identity=id5[:])
                nm2 = nmp.tile([P, F2], f32, tag="nm2")
                nc.vector.tensor_copy(out=nm2[:], in_=ps_tr2[:])
                rows = min(P, S - t * P)
                if rows > 0:
                    nc.sync.dma_start(out=tb2_loc[t * P:t * P + rows, :],
                                      in_=nm2[:rows, 0:RC2])
                nc.vector.tensor_copy(out=aD2[:, t:t + 1], in_=nm2[:, 4:5])

        for j in range(4):
            st = sched["chunk_starts"][j]
            sz = sched["chunk_rows"][j]
            bs = sched["chunk_bases"][j]
            nc.gpsimd.collective_compute(
                "AllGather", mybir.AluOpType.bypass,
                replica_groups=[list(range(C))],
                ins=[tb2_loc[st:st + sz, :].opt()],
                outs=[tb2_full[bs:bs + C * sz, :].opt()],
            )

        # ---- STEP E: layer-2 aggregation ----
        def l2_out(t, o):
            rows = min(P, S - t * P)
            if rows > 0:
                nc.sync.dma_start(out=out_d[t * P:t * P + rows, :], in_=o[:rows, :])

        for t in range(NT):
            gat_tile(t, tb2_full, RC2, 3, aD2, l2_out)

    nc.compile()
    return nc


def _prepare(inputs):
    x = np.asarray(inputs["x"], dtype=np.float32)
    edge_index = np.asarray(inputs["edge_index"])
    orders, idx_cores, sched = _preprocess(edge_index)
    Wbig, bbig, P2 = _fold_weights(
        np.asarray(inputs["W_lin"], np.float32), np.asarray(inputs["b_lin"], np.float32),
        np.asarray(inputs["W1"], np.float32), np.asarray(inputs["att_src1"], np.float32),
        np.asarray(inputs["att_dst1"], np.float32), np.asarray(inputs["W2"], np.float32),
        np.asarray(inputs["att_src2"], np.float32), np.asarray(inputs["att_dst2"], np.float32))
    b1 = np.asarray(inputs["b1"], np.float32)
    b2 = np.asarray(inputs["b2"], np.float32)

    in_maps = []
    for c in range(C):
        xs = np.zeros((768, S_PAD), np.float32)
        xs[:, :S] = x[orders[c]].T
        in_maps.append({
            "xT": np.ascontiguousarray(xs),
            "Wbig": Wbig, "bbig": bbig[:, None].copy(), "P2": P2,
            "b1": b1, "b2": b2,
            "idx": idx_cores[c],
        })
    return orders, sched, in_maps


def kernel(**inputs):
    from concourse.bass_utils import run_bass_kernel_spmd

    orders, sched, in_maps = _prepare(inputs)
    key = ("prog", tuple(sched["Dt"]))
    if key not in _CACHE:
        _CACHE[key] = _build_program(sched)
    nc = _CACHE[key]

    res = run_bass_kernel_spmd(nc, in_maps, core_ids=list(range(C)), trace=False)
    out = np.empty((N, 3), np.float32)
    for c in range(C):
        out[orders[c]] = res.results[c]["out"]
    return out


# ---------------------------------------------------------------------------
# numpy golden model of the device pipeline (for test harnesses)
def golden(**inputs):
    x = np.asarray(inputs["x"], np.float32)
    orders, idx_cores, sched = _preprocess(np.asarray(inputs["edge_index"]))
    Wbig, bbig, P2 = _fold_weights(
        np.asarray(inputs["W_lin"], np.float32), np.asarray(inputs["b_lin"], np.float32),
        np.asarray(inputs["W1"], np.float32), np.asarray(inputs["att_src1"], np.float32),
        np.asarray(inputs["att_dst1"], np.float32), np.asarray(inputs["W2"], np.float32),
        np.asarray(inputs["att_src2"], np.float32), np.asarray(inputs["att_dst2"], np.float32))
    b1 = np.asarray(inputs["b1"], np.float32)
    b2 = np.asarray(inputs["b2"], np.float32)
    Dt = sched["Dt"]
    toff = sched["toff"]

    # tables
    tb1 = np.zeros((TR, RC1), np.float32)
    tb2 = np.zeros((TR, RC2), np.float32)
    tb1[DUMMY, 64:66] = ALPHA_PAD
    tb2[DUMMY, 3] = ALPHA_PAD

    fms = []
    for c in range(C):
        fm = (x[orders[c]] @ Wbig + bbig).astype(np.float32)      # [S, 68]
        fms.append(fm)

    # slot mapping identical to _preprocess
    rank = np.arange(S)
    j_of = np.minimum(rank // 3200, 3)
    sizes = np.array(sched["chunk_rows"]); starts = np.array(sched["chunk_starts"])
    bases = np.array(sched["chunk_bases"])
    for c in range(C):
        slots = bases[j_of] + c * sizes[j_of] + (rank - starts[j_of])
        tb1[slots] = fms[c][:, 0:RC1]

    def leaky(v):
        return np.where(v >= 0, v, NEG_SLOPE * v)

    out = np.empty((N, 3), np.float32)
    h1T = {}
    for c in range(C):
        idxc = idx_cores[c]
        aD1 = np.zeros((S_PAD, 2), np.float32)
        aD1[:S] = fms[c][:, 66:68]
        h1c = np.zeros((S_PAD, 64), np.float32)
        for t in range(NT):
            g = tb1[idxc[:, toff[t]:toff[t + 1]]]                  # [128, D, 66]
            asv = g[:, :, 64:66]                                   # [128, D, 2]
            adv = aD1[t * P:(t + 1) * P][:, None, :]               # [128, 1, 2]
            e = leaky(asv + adv)                                   # [128, D, 2]
            m = e.max(axis=1, keepdims=True)
            pp = np.exp(e - m)
            den = pp.sum(axis=1, keepdims=True)
            inv = 1.0 / den
            hh = g[:, :, 0:64].reshape(P, -1, 2, CH)
            o = (hh * pp[:, :, :, None]).sum(axis=1) * inv[:, 0, :, None]
            h1c[t * P:(t + 1) * P] = o.reshape(P, 64) + b1
        h1T[c] = h1c
        fm2 = h1c[:S] @ P2                                         # [S, 5]
        slots = bases[j_of] + c * sizes[j_of] + (rank - starts[j_of])
        tb2[slots] = fm2[:, 0:RC2]
        h1T[c] = (h1c, fm2)

    for c in range(C):
        idxc = idx_cores[c]
        h1c, fm2 = h1T[c]
        aD2 = np.zeros((S_PAD,), np.float32)
        aD2[:S] = fm2[:, 4]
        oc = np.zeros((S, 3), np.float32)
        for t in range(NT):
            g = tb2[idxc[:, toff[t]:toff[t + 1]]]                  # [128, D, 4]
            e = leaky(g[:, :, 3] + aD2[t * P:(t + 1) * P][:, None])
            m = e.max(axis=1, keepdims=True)
            pp = np.exp(e - m)
            den = pp.sum(axis=1, keepdims=True)
            o = (g[:, :, 0:3] * pp[:, :, None]).sum(axis=1) / den
            rows = min(P, S - t * P)
            oc[t * P:t * P + rows] = o[:rows] + b2
        out[orders[c]] = oc
    return out


# revision 11
# speedup vs baseline: 1.0307x; 1.0307x over previous
"""Trainium2 Bass kernel for a 2-layer GAT (nn_GAT_1236950581751).

Strategy (8 NeuronCores, SPMD, one program):
  - Nodes are sharded contiguously: core c owns nodes [c*12500, (c+1)*12500),
    locally reordered by in-degree (descending) so that 128-node dst tiles
    have near-uniform degree.
  - Host folds weights: layer-1 needs only h1pre = x @ (W_lin@W1) + b_lin@W1
    plus the 4 attention projections -> one [768, 68] matmul per node.
  - Device: fm = Wbig^T @ x^T (feature-major), PE-transpose to node-major,
    write a local node table [12500, 66] (h1pre + alpha_src), AllGather to the
    full table [100001, 66] (row 100000 is a dummy row used for ELL padding:
    h = 0, alpha_src = -300 so exp() underflows to 0).
  - Per dst tile (128 nodes, ELL with D_t slot-columns): one indirect DMA per
    slot column gathers table[idx[p, j]] into SBUF; DVE computes the
    segment-softmax (max-shifted exp) and the alpha-weighted sum along the
    slot axis; bias add; PE-transpose into a feature-major h1 buffer.
  - Layer 2 repeats the same structure with a [100001, 4] table
    (h2pre(3) + alpha_src2) and 1 head; output written per tile.
  - Host un-permutes the 8 output shards into the full [100000, 3] result.
"""

import numpy as np

N = 100000
C = 8                 # cores
S = N // C            # 12500 nodes per shard
P = 128
NT = (S + P - 1) // P  # 98 tiles per core
S_PAD = NT * P         # 12544
DUMMY = N              # dummy table row
TR = N + 1             # table rows
RC1 = 66               # layer-1 table row: h1pre(64) + alpha_src(2)
RC2 = 4                # layer-2 table row: h2pre(3) + alpha_src2(1)
ALPHA_PAD = -300.0
NEG_SLOPE = 0.2
H = 2                  # layer-1 heads
CH = 32                # channels per head
F1 = 68                # fm channels: 64 h1pre + 2 a_src + 2 a_dst
F2 = 5                 # fm2 channels: 3 h2pre + 1 a_src + 1 a_dst
KX = 6                 # 768 / 128 contraction chunks
NCHUNK = 512           # matmul free-dim tile

_CACHE = {}


def _fold_weights(W_lin, b_lin, W1, att_src1, att_dst1, W2, att_src2, att_dst2):
    Wf = (W_lin.astype(np.float64) @ W1.astype(np.float64))
    bf = (b_lin.astype(np.float64) @ W1.astype(np.float64))
    cols = [Wf]
    bb = [bf]
    for att in (att_src1, att_dst1):
        for h in range(H):
            a = att[h].astype(np.float64)
            cols.append((Wf[:, CH * h:CH * (h + 1)] @ a)[:, None])
            bb.append(np.array([bf[CH * h:CH * (h + 1)] @ a]))
    Wbig = np.concatenate(cols, axis=1).astype(np.float32)        # [768, 68]
    bbig = np.concatenate(bb).astype(np.float32)                  # [68]
    W2l = W2.astype(np.float64)
    P2 = np.concatenate(
        [W2l, (W2l @ att_src2[0].astype(np.float64))[:, None],
         (W2l @ att_dst2[0].astype(np.float64))[:, None]], axis=1
    ).astype(np.float32)                                          # [64, 5]
    return Wbig, bbig, P2


def _preprocess(edge_index):
    """Static graph preprocessing -> per-core idx arrays + shared schedule.

    The appended self-loop of every node is NOT put in the ELL; it is served
    on-device by an affine read of the core's own local table rows (the
    "self column"). Natural (v, v) edges in edge_index stay in the ELL.
    """
    src = np.asarray(edge_index[0], dtype=np.int64)
    dst = np.asarray(edge_index[1], dtype=np.int64)
    deg = np.bincount(dst, minlength=N).astype(np.int64)

    # CSR over dst
    order_e = np.argsort(dst, kind="stable")
    src_by_dst = src[order_e]
    rowptr = np.zeros(N + 1, np.int64)
    rowptr[1:] = np.cumsum(deg)

    # per-shard degree-descending node order
    orders = np.empty((C, S), np.int64)
    for c in range(C):
        nodes = np.arange(c * S, (c + 1) * S)
        orders[c] = nodes[np.argsort(-deg[nodes], kind="stable")]
    rank = np.empty(N, np.int64)
    for c in range(C):
        rank[orders[c]] = np.arange(S)
    shard_of = np.arange(N) // S

    # chunk-major table slot numbering (4 collective chunks, tile aligned)
    chunk_tiles = [25, 25, 25, 23]
    starts = np.array([0, 3200, 6400, 9600], np.int64)
    sizes = np.array([3200, 3200, 3200, 2900], np.int64)
    bases = np.array([0, 25600, 51200, 76800], np.int64)
    j_of = np.minimum(rank // 3200, 3)
    slot = bases[j_of] + shard_of * sizes[j_of] + (rank - starts[j_of])

    # shared per-tile max-degree schedule
    Dt = np.zeros(NT, np.int64)
    for c in range(C):
        dpad = np.zeros(S_PAD, np.int64)
        dpad[:S] = deg[orders[c]]
        Dt = np.maximum(Dt, dpad.reshape(NT, P).max(1))
    Dt = Dt.astype(np.int64)
    toff = np.zeros(NT + 1, np.int64)
    toff[1:] = np.cumsum(Dt)
    TOT = int(toff[-1])

    # per-core ELL index array [128, TOT] int32 (slot ids; DUMMY padding)
    Dmax = int(Dt.max())
    col = np.arange(Dmax)[None, :]
    idx_cores = []
    for c in range(C):
        nodes = orders[c]
        counts = deg[nodes]
        ell = np.full((S_PAD, Dmax), DUMMY, np.int32)
        mask = col < counts[:, None]
        pos = (rowptr[nodes][:, None] + col)[mask]
        rr, cc = np.nonzero(mask)
        ell[rr, cc] = slot[src_by_dst[pos]].astype(np.int32)
        idxc = np.empty((P, TOT), np.int32)
        for t in range(NT):
            idxc[:, toff[t]:toff[t + 1]] = ell[t * P:(t + 1) * P, :Dt[t]]
        idx_cores.append(idxc)

    sched = {
        "Dt": [int(d) for d in Dt],
        "toff": [int(o) for o in toff],
        "TOT": TOT,
        "chunk_tiles": chunk_tiles,
        "chunk_rows": [int(x) for x in sizes],
        "chunk_starts": [int(x) for x in starts],
        "chunk_bases": [int(x) for x in bases],
    }
    return orders, idx_cores, sched


def _build_program(sched):
    import concourse.bass as bass
    import concourse.mybir as mybir
    import concourse.tile as tile
    from concourse import bacc
    from concourse.masks import make_identity

    f32 = mybir.dt.float32
    i32 = mybir.dt.int32
    Dt = sched["Dt"]
    toff = sched["toff"]
    TOT = sched["TOT"]

    nc = bacc.Bacc("TRN2", target_bir_lowering=False, debug=False,
                   enable_asserts=False, num_devices=C)

    xT = nc.dram_tensor("xT", [768, S_PAD], f32, kind="ExternalInput")
    Wbig_d = nc.dram_tensor("Wbig", [768, F1], f32, kind="ExternalInput")
    bbig_d = nc.dram_tensor("bbig", [F1, 1], f32, kind="ExternalInput")
    P2_d = nc.dram_tensor("P2", [64, F2], f32, kind="ExternalInput")
    b1_d = nc.dram_tensor("b1", [64], f32, kind="ExternalInput")
    b2_d = nc.dram_tensor("b2", [3], f32, kind="ExternalInput")
    idx_d = nc.dram_tensor("idx", [P, TOT], i32, kind="ExternalInput")
    out_d = nc.dram_tensor("out", [S, 3], f32, kind="ExternalOutput")

    # local tables split per collective chunk so each AllGather only waits
    # for its own chunk's rows
    tb1_locs = [nc.dram_tensor(f"tb1_loc{j}", [sched["chunk_rows"][j], RC1], f32,
                               kind="Internal") for j in range(4)]
    tb1_full = nc.dram_tensor("tb1_full", [TR, RC1], f32, kind="Internal",
                              addr_space="Shared")
    tb2_locs = [nc.dram_tensor(f"tb2_loc{j}", [sched["chunk_rows"][j], RC2], f32,
                               kind="Internal") for j in range(4)]
    tb2_full = nc.dram_tensor("tb2_full", [TR, RC2], f32, kind="Internal",
                              addr_space="Shared")
    cstarts = sched["chunk_starts"]

    def loc_write(locs, row0, rows, src_ap):
        j = min(row0 // 3200, 3)
        nc.sync.dma_start(out=locs[j][row0 - cstarts[j]:row0 - cstarts[j] + rows, :],
                          in_=src_ap)

    # matmul N chunks
    chunks = []
    c0 = 0
    while c0 < S_PAD:
        cw = min(NCHUNK, S_PAD - c0)
        chunks.append((c0, cw))
        c0 += cw

    from contextlib import ExitStack

    with tile.TileContext(nc) as tc, ExitStack() as stack:
        const = stack.enter_context(tc.tile_pool(name="const", bufs=1))
        big = stack.enter_context(tc.tile_pool(name="big", bufs=1))
        io = stack.enter_context(tc.tile_pool(name="io", bufs=3))
        fmp = stack.enter_context(tc.tile_pool(name="fmp", bufs=2))
        nmp = stack.enter_context(tc.tile_pool(name="nmp", bufs=3))
        gp = stack.enter_context(tc.tile_pool(name="gp", bufs=4))
        wk = stack.enter_context(tc.tile_pool(name="wk", bufs=4))
        ps = stack.enter_context(tc.tile_pool(name="ps", bufs=2, space="PSUM"))

        # ---- constants ----
        wtiles = const.tile([P, KX, F1], f32)
        for k in range(KX):
            nc.sync.dma_start(out=wtiles[:, k, :], in_=Wbig_d[k * P:(k + 1) * P, :])
        bbig_sb = const.tile([F1, 1], f32)
        nc.sync.dma_start(out=bbig_sb[:], in_=bbig_d[:])
        p2_sb = const.tile([64, F2], f32)
        nc.sync.dma_start(out=p2_sb[:], in_=P2_d[:])
        b1_bc = const.tile([P, 64], f32)
        nc.sync.dma_start(out=b1_bc[:], in_=bass.AP(
            tensor=b1_d, offset=0, ap=[[0, P], [1, 64]]))
        b2_bc = const.tile([P, 3], f32)
        nc.sync.dma_start(out=b2_bc[:], in_=bass.AP(
            tensor=b2_d, offset=0, ap=[[0, P], [1, 3]]))
        id68 = const.tile([F1, F1], f32)
        make_identity(nc, id68[:])
        id128 = const.tile([P, P], f32)
        make_identity(nc, id128[:])
        id5 = const.tile([F2, F2], f32)
        make_identity(nc, id5[:])

        idx_all = big.tile([P, TOT], i32)
        nc.sync.dma_start(out=idx_all[:], in_=idx_d[:])
        h1T_all = big.tile([64, S_PAD], f32)
        aD1 = big.tile([P, 2 * NT], f32)
        aD2 = big.tile([P, NT], f32)

        # dummy rows
        dummy1 = const.tile([1, RC1], f32)
        nc.vector.memset(dummy1[:], 0.0)
        nc.vector.memset(dummy1[:, 64:66], ALPHA_PAD)
        nc.sync.dma_start(out=tb1_full[DUMMY:DUMMY + 1, :], in_=dummy1[:])
        dummy2 = const.tile([1, RC2], f32)
        nc.vector.memset(dummy2[:], 0.0)
        nc.vector.memset(dummy2[:, 3:4], ALPHA_PAD)
        nc.sync.dma_start(out=tb2_full[DUMMY:DUMMY + 1, :], in_=dummy2[:])

        # ---- STEP A: fm = Wbig^T @ x^T, transpose, write local table ----
        scopeA = nc.named_scope("stepA"); scopeA.__enter__()
        t_idx = 0
        for (cst, cw) in chunks:
            ps_fm = ps.tile([F1, cw], f32, tag="fm")
            for k in range(KX):
                xt = io.tile([P, cw], f32, tag="xt")
                nc.sync.dma_start(out=xt[:], in_=xT[k * P:(k + 1) * P, cst:cst + cw])
                nc.tensor.matmul(out=ps_fm[:], lhsT=wtiles[:, k, :], rhs=xt[:],
                                 start=(k == 0), stop=(k == KX - 1))
            fm_sb = fmp.tile([F1, cw], f32, tag="fm_sb")
            nc.vector.tensor_scalar(out=fm_sb[:], in0=ps_fm[:],
                                    scalar1=bbig_sb[:, 0:1], scalar2=None,
                                    op0=mybir.AluOpType.add)
            for sub in range(cw // P):
                t = t_idx
                t_idx += 1
                ps_tr = ps.tile([P, F1], f32, tag="tr")
                nc.tensor.transpose(out=ps_tr[:], in_=fm_sb[:, sub * P:(sub + 1) * P],
                                    identity=id68[:])
                nm = nmp.tile([P, F1], f32, tag="nm")
                nc.vector.tensor_copy(out=nm[:], in_=ps_tr[:])
                rows = min(P, S - t * P)
                if rows > 0:
                    loc_write(tb1_locs, t * P, rows, nm[:rows, 0:RC1])
                nc.vector.tensor_copy(out=aD1[:, 2 * t:2 * t + 2], in_=nm[:, 66:68])

        # ---- STEP B: chunked AllGather of table1 ----
        scopeA.__exit__(None, None, None)
        scopeB = nc.named_scope("ag1"); scopeB.__enter__()
        for j in range(4):
            sz = sched["chunk_rows"][j]
            bs = sched["chunk_bases"][j]
            nc.gpsimd.collective_compute(
                "AllGather", mybir.AluOpType.bypass,
                replica_groups=[list(range(C))],
                ins=[tb1_locs[j][:].opt()],
                outs=[tb1_full[bs:bs + C * sz, :].opt()],
            )

        # ---- STEP C: layer-1 aggregation per tile ----
        scopeB.__exit__(None, None, None)
        def gat_tile(t, tbl, tbl_locs, rc, nch, aD, out_cb):
            """Gather + segment softmax + weighted sum for dst tile t.

            Slots [0, D) are gathered via indirect DMA; slot D (the appended
            self-loop) is an affine read of this core's local table rows.
            """
            D = Dt[t]
            DG = D + 1
            g = gp.tile([P, DG * rc], f32, tag=f"g{rc}",
                        padded_shape=[P, (max(Dt) + 1) * rc])
            for j in range(D):
                nc.gpsimd.indirect_dma_start(
                    out=g[:, j * rc:(j + 1) * rc],
                    out_offset=None,
                    in_=tbl[:],
                    in_offset=bass.IndirectOffsetOnAxis(
                        ap=idx_all[:, toff[t] + j:toff[t] + j + 1], axis=0),
                )
            jch = min((t * P) // 3200, 3)
            lrow = t * P - cstarts[jch]
            srows = min(P, S - t * P)
            nc.sync.dma_start(out=g[:srows, D * rc:DG * rc],
                              in_=tbl_locs[jch][lrow:lrow + srows, :])
            g3 = g[:].rearrange("p (d c) -> p d c", c=rc)
            if nch == 64:  # layer 1, H=2 heads
                asv = g3[:, :, 64:66].transpose([0, 2, 1])        # [P,2,DG]
                adv = aD[:, 2 * t:2 * t + 2].unsqueeze(2).to_broadcast([P, 2, DG])
                e = wk.tile([P, 2, DG], f32, tag="e")
                nc.vector.tensor_tensor(out=e[:], in0=asv, in1=adv,
                                        op=mybir.AluOpType.add)
                tmp = wk.tile([P, 2, DG], f32, tag="tmp")
                nc.vector.tensor_scalar_mul(tmp[:], e[:], NEG_SLOPE)
                nc.vector.tensor_tensor(out=e[:], in0=e[:], in1=tmp[:],
                                        op=mybir.AluOpType.max)
                negm = wk.tile([P, 2], f32, tag="negm")
                nc.vector.tensor_reduce(out=negm[:], in_=e[:],
                                        axis=mybir.AxisListType.X,
                                        op=mybir.AluOpType.max, negate=True)
                pp = wk.tile([P, 2, DG], f32, tag="pp")
                for h in range(2):
                    nc.scalar.activation(out=pp[:, h, :], in_=e[:, h, :],
                                         func=mybir.ActivationFunctionType.Exp,
                                         bias=negm[:, h:h + 1], scale=1.0)
                den = wk.tile([P, 2], f32, tag="den")
                nc.vector.tensor_reduce(out=den[:], in_=pp[:],
                                        axis=mybir.AxisListType.X,
                                        op=mybir.AluOpType.add)
                inv = wk.tile([P, 2], f32, tag="inv")
                nc.vector.reciprocal(inv[:], den[:])
                hv = g3[:, :, 0:64].rearrange("p d (h c) -> p d h c", h=2)
                pv = pp[:].transpose([0, 2, 1]).unsqueeze(3).to_broadcast(
                    [P, DG, 2, CH])
                nc.vector.tensor_tensor(out=hv, in0=hv, in1=pv,
                                        op=mybir.AluOpType.mult)
                o = wk.tile([P, 2, CH], f32, tag="o")
                nc.vector.tensor_reduce(out=o[:], in_=hv.transpose([0, 2, 3, 1]),
                                        axis=mybir.AxisListType.X,
                                        op=mybir.AluOpType.add)
                invv = inv[:].unsqueeze(2).to_broadcast([P, 2, CH])
                nc.vector.tensor_tensor(out=o[:], in0=o[:], in1=invv,
                                        op=mybir.AluOpType.mult)
                h1 = wk.tile([P, 64], f32, tag="h1")
                nc.vector.tensor_tensor(out=h1[:], in0=o[:].rearrange("p h c -> p (h c)"),
                                        in1=b1_bc[:], op=mybir.AluOpType.add)
                out_cb(t, h1)
            else:  # layer 2, 1 head, 3 channels
                as2 = g3[:, :, 3:4].squeeze(2)                    # [P, DG]
                e = wk.tile([P, DG], f32, tag="e2")
                nc.vector.tensor_scalar(out=e[:], in0=as2, scalar1=aD[:, t:t + 1],
                                        scalar2=None, op0=mybir.AluOpType.add)
                tmp = wk.tile([P, DG], f32, tag="tmp2")
                nc.vector.tensor_scalar_mul(tmp[:], e[:], NEG_SLOPE)
                nc.vector.tensor_tensor(out=e[:], in0=e[:], in1=tmp[:],
                                        op=mybir.AluOpType.max)
                negm = wk.tile([P, 1], f32, tag="negm2")
                nc.vector.tensor_reduce(out=negm[:], in_=e[:],
                                        axis=mybir.AxisListType.X,
                                        op=mybir.AluOpType.max, negate=True)
                pp = wk.tile([P, DG], f32, tag="pp2")
                nc.scalar.activation(out=pp[:], in_=e[:],
                                     func=mybir.ActivationFunctionType.Exp,
                                     bias=negm[:, 0:1], scale=1.0)
                den = wk.tile([P, 1], f32, tag="den2")
                nc.vector.tensor_reduce(out=den[:], in_=pp[:],
                                        axis=mybir.AxisListType.X,
                                        op=mybir.AluOpType.add)
                inv = wk.tile([P, 1], f32, tag="inv2")
                nc.vector.reciprocal(inv[:], den[:])
                hv = g3[:, :, 0:3]
                pv = pp[:].unsqueeze(2).to_broadcast([P, DG, 3])
                nc.vector.tensor_tensor(out=hv, in0=hv, in1=pv,
                                        op=mybir.AluOpType.mult)
                o = wk.tile([P, 3], f32, tag="o2")
                nc.vector.tensor_reduce(out=o[:], in_=hv.transpose([0, 2, 1]),
                                        axis=mybir.AxisListType.X,
                                        op=mybir.AluOpType.add)
                nc.vector.tensor_scalar(out=o[:], in0=o[:], scalar1=inv[:, 0:1],
                                        scalar2=None, op0=mybir.AluOpType.mult)
                nc.vector.tensor_tensor(out=o[:], in0=o[:], in1=b2_bc[:],
                                        op=mybir.AluOpType.add)
                out_cb(t, o)

        def l1_out(t, h1):
            ps_h1t = ps.tile([64, P], f32, tag="h1t")
            nc.tensor.transpose(out=ps_h1t[:], in_=h1[:], identity=id128[:])
            nc.vector.tensor_copy(out=h1T_all[:, t * P:(t + 1) * P], in_=ps_h1t[:])

        scopeC = nc.named_scope("layer1"); scopeC.__enter__()
        for t in range(NT):
            gat_tile(t, tb1_full, tb1_locs, RC1, 64, aD1, l1_out)
        scopeC.__exit__(None, None, None)

        # ---- STEP D: layer-2 node projections + table2 ----
        scopeD = nc.named_scope("stepD"); scopeD.__enter__()
        t_idx = 0
        for (cst, cw) in chunks:
            ps2 = ps.tile([F2, cw], f32, tag="fm")
            nc.tensor.matmul(out=ps2[:], lhsT=p2_sb[:], rhs=h1T_all[:, cst:cst + cw],
                             start=True, stop=True)
            fm2 = fmp.tile([F2, cw], f32, tag="fm2_sb")
            nc.vector.tensor_copy(out=fm2[:], in_=ps2[:])
            for sub in range(cw // P):
                t = t_idx
                t_idx += 1
                ps_tr2 = ps.tile([P, F2], f32, tag="tr")
                nc.tensor.transpose(out=ps_tr2[:], in_=fm2[:, sub * P:(sub + 1) * P],
                                    identity=id5[:])
                nm2 = nmp.tile([P, F2], f32, tag="nm2")
                nc.vector.tensor_copy(out=nm2[:], in_=ps_tr2[:])
                rows = min(P, S - t * P)
                if rows > 0:
                    loc_write(tb2_locs, t * P, rows, nm2[:rows, 0:RC2])
                nc.vector.tensor_copy(out=aD2[:, t:t + 1], in_=nm2[:, 4:5])

        scopeD.__exit__(None, None, None)
        scopeG = nc.named_scope("ag2"); scopeG.__enter__()
        for j in range(4):
            sz = sched["chunk_rows"][j]
            bs = sched["chunk_bases"][j]
            nc.gpsimd.collective_compute(
                "AllGather", mybir.AluOpType.bypass,
                replica_groups=[list(range(C))],
                ins=[tb2_locs[j][:].opt()],
                outs=[tb2_full[bs:bs + C * sz, :].opt()],
            )
        scopeG.__exit__(None, None, None)

        # ---- STEP E: layer-2 aggregation ----
        def l2_out(t, o):
            rows = min(P, S - t * P)
            if rows > 0:
                nc.sync.dma_start(out=out_d[t * P:t * P + rows, :], in_=o[:rows, :])

        scopeE = nc.named_scope("layer2"); scopeE.__enter__()
        for t in range(NT):
            gat_tile(t, tb2_full, tb2_locs, RC2, 3, aD2, l2_out)
        scopeE.__exit__(None, None, None)

    nc.compile()
    return nc


def _prepare(inputs):
    x = np.asarray(inputs["x"], dtype=np.float32)
    edge_index = np.asarray(inputs["edge_index"])
    orders, idx_cores, sched = _preprocess(edge_index)
    Wbig, bbig, P2 = _fold_weights(
        np.asarray(inputs["W_lin"], np.float32), np.asarray(inputs["b_lin"], np.float32),
        np.asarray(inputs["W1"], np.float32), np.asarray(inputs["att_src1"], np.float32),
        np.asarray(inputs["att_dst1"], np.float32), np.asarray(inputs["W2"], np.float32),
        np.asarray(inputs["att_src2"], np.float32), np.asarray(inputs["att_dst2"], np.float32))
    b1 = np.asarray(inputs["b1"], np.float32)
    b2 = np.asarray(inputs["b2"], np.float32)

    in_maps = []
    for c in range(C):
        xs = np.zeros((768, S_PAD), np.float32)
        xs[:, :S] = x[orders[c]].T
        in_maps.append({
            "xT": np.ascontiguousarray(xs),
            "Wbig": Wbig, "bbig": bbig[:, None].copy(), "P2": P2,
            "b1": b1, "b2": b2,
            "idx": idx_cores[c],
        })
    return orders, sched, in_maps


def kernel(**inputs):
    import time
    from concourse.bass_utils import run_bass_kernel_spmd

    orders, sched, in_maps = _prepare(inputs)
    key = ("prog", tuple(sched["Dt"]))
    if key not in _CACHE:
        _CACHE[key] = _build_program(sched)
    nc = _CACHE[key]

    res = None
    for attempt in range(3):
        try:
            res = run_bass_kernel_spmd(nc, in_maps, core_ids=list(range(C)),
                                       trace=False)
            break
        except Exception:
            # transient NRT_EXEC_UNIT_UNRECOVERABLE wedges recover after ~60s
            if attempt == 2:
                raise
            time.sleep(75)
    out = np.empty((N, 3), np.float32)
    for c in range(C):
        out[orders[c]] = res.results[c]["out"]
    return out


# ---------------------------------------------------------------------------
# numpy golden model of the device pipeline (for test harnesses)
def golden(**inputs):
    x = np.asarray(inputs["x"], np.float32)
    orders, idx_cores, sched = _preprocess(np.asarray(inputs["edge_index"]))
    Wbig, bbig, P2 = _fold_weights(
        np.asarray(inputs["W_lin"], np.float32), np.asarray(inputs["b_lin"], np.float32),
        np.asarray(inputs["W1"], np.float32), np.asarray(inputs["att_src1"], np.float32),
        np.asarray(inputs["att_dst1"], np.float32), np.asarray(inputs["W2"], np.float32),
        np.asarray(inputs["att_src2"], np.float32), np.asarray(inputs["att_dst2"], np.float32))
    b1 = np.asarray(inputs["b1"], np.float32)
    b2 = np.asarray(inputs["b2"], np.float32)
    Dt = sched["Dt"]
    toff = sched["toff"]

    # tables
    tb1 = np.zeros((TR, RC1), np.float32)
    tb2 = np.zeros((TR, RC2), np.float32)
    tb1[DUMMY, 64:66] = ALPHA_PAD
    tb2[DUMMY, 3] = ALPHA_PAD

    fms = []
    for c in range(C):
        fm = (x[orders[c]] @ Wbig + bbig).astype(np.float32)      # [S, 68]
        fms.append(fm)

    # slot mapping identical to _preprocess
    rank = np.arange(S)
    j_of = np.minimum(rank // 3200, 3)
    sizes = np.array(sched["chunk_rows"]); starts = np.array(sched["chunk_starts"])
    bases = np.array(sched["chunk_bases"])
    for c in range(C):
        slots = bases[j_of] + c * sizes[j_of] + (rank - starts[j_of])
        tb1[slots] = fms[c][:, 0:RC1]

    def leaky(v):
        return np.where(v >= 0, v, NEG_SLOPE * v)

    out = np.empty((N, 3), np.float32)
    h1T = {}
    for c in range(C):
        idxc = idx_cores[c]
        aD1 = np.zeros((S_PAD, 2), np.float32)
        aD1[:S] = fms[c][:, 66:68]
        selfrows1 = np.zeros((S_PAD, RC1), np.float32)
        selfrows1[:S] = fms[c][:, 0:RC1]
        h1c = np.zeros((S_PAD, 64), np.float32)
        for t in range(NT):
            gg = tb1[idxc[:, toff[t]:toff[t + 1]]]                 # [128, D, 66]
            g = np.concatenate([gg, selfrows1[t * P:(t + 1) * P][:, None, :]], 1)
            asv = g[:, :, 64:66]
            adv = aD1[t * P:(t + 1) * P][:, None, :]
            e = leaky(asv + adv)
            m = e.max(axis=1, keepdims=True)
            pp = np.exp(e - m)
            den = pp.sum(axis=1, keepdims=True)
            inv = 1.0 / den
            hh = g[:, :, 0:64].reshape(P, -1, 2, CH)
            o = (hh * pp[:, :, :, None]).sum(axis=1) * inv[:, 0, :, None]
            h1c[t * P:(t + 1) * P] = o.reshape(P, 64) + b1
        h1T[c] = h1c
        fm2 = h1c[:S] @ P2                                         # [S, 5]
        slots = bases[j_of] + c * sizes[j_of] + (rank - starts[j_of])
        tb2[slots] = fm2[:, 0:RC2]
        h1T[c] = (h1c, fm2)

    for c in range(C):
        idxc = idx_cores[c]
        h1c, fm2 = h1T[c]
        aD2 = np.zeros((S_PAD,), np.float32)
        aD2[:S] = fm2[:, 4]
        selfrows2 = np.zeros((S_PAD, RC2), np.float32)
        selfrows2[:S] = fm2[:, 0:RC2]
        oc = np.zeros((S, 3), np.float32)
        for t in range(NT):
            gg = tb2[idxc[:, toff[t]:toff[t + 1]]]                 # [128, D, 4]
            g = np.concatenate([gg, selfrows2[t * P:(t + 1) * P][:, None, :]], 1)
            e = leaky(g[:, :, 3] + aD2[t * P:(t + 1) * P][:, None])
            m = e.max(axis=1, keepdims=True)
            pp = np.exp(e - m)
            den = pp.sum(axis=1, keepdims=True)
            o = (g[:, :, 0:3] * pp[:, :, None]).sum(axis=1) / den
            rows = min(P, S - t * P)
            oc[t * P:t * P + rows] = o[:rows] + b2
        out[orders[c]] = oc
    return out
